# revision 32
# baseline (speedup 1.0000x reference)
"""Trainium2 Bass kernel for nn_BandSplitDCTFilter (v10 redesign).

Math: out_c = C1 (Z_c) C2^T - S1 (Z_c) S2^T, Z_c = (A x_c A^T) .* W_eff_c,
then y = x_out @ proj_w^T and LayerNorm (same collapse as v9).

v10 eliminates both DRAM round-trip pivots of v9:
  - The F-h stage uses the DATA as the stationary operand (lhsT = x chunk,
    rhs = A^T), so its output lands transposed: partitions become (c2, w).
    The first pivot disappears into the matmul.
  - F-w uses blkdiag(A^T, A^T) over the (c2, w) partitions.
  - I-l again uses data-as-stationary (lhsT = Z chunk, rhs = [CS2|CS2]
    parity-masked), landing (j2, k) on partitions.
  - I-k contracts k per j2 block with blkdiag(C1)/blkdiag(-S1) PSUM
    accumulation; cos/sin picked from Vbuf by strided rhs APs.
  - One remaining pivot (spatial->channel, before proj) rides DRAM:
    contiguous 2.9us store + strided reload (128B runs).
All matmuls bf16 (fp32 PSUM). weff and the output are bf16 (validated:
rel err 5.4e-3 vs 2e-2 budget). Output y_d is [128, (t 32, d 256)] with
row n = t*128 + p, unpermuted on host.

Sharding: pure data-parallel, one sample per core (B=8 = 8 cores).
"""

import os

os.environ.setdefault("JAX_PLATFORMS", "axon,cpu")

import numpy as np
import ml_dtypes

import bass_rust
import concourse.bass as bass
import concourse.mybir as mybir
from concourse.tile import TileContext, ScopedClock
from concourse.bass_utils import run_bass_kernel_spmd

# ---------------------------------------------------------------------------
# Workarounds: this container's walrus rejects >1 sync wait per instruction.
# ---------------------------------------------------------------------------

_wait_ctr = 0


def _split_multi_waits(nc, max_waits=1):
    global _wait_ctr
    for f in nc.m.functions:
        for bb in f.blocks:
            out = []
            dirty = False
            for ins in bb.instructions:
                si = ins.sync_info
                if si is not None and len(si.on_wait) > max_waits:
                    waits = list(si.on_wait)
                    for w in waits[:-max_waits]:
                        _wait_ctr += 1
                        nop = bass_rust.InstNoOp(name=f"I-waitsplit-{_wait_ctr}")
                        nop.engine = ins.engine
                        nop.sync_info = mybir.SyncInfo(on_wait=[w], on_update=[])
                        out.append(nop)
                    ins.sync_info = mybir.SyncInfo(
                        on_wait=waits[-max_waits:], on_update=list(si.on_update)
                    )
                    dirty = True
                out.append(ins)
            if dirty:
                bb.instructions = out


def _patched_drain_and_barrier(self, tick_clock, wait_clock):
    nc = self.nc
    probe = nc.sync.nop(nofuse=True)
    wait_clock.add_sem_waits(probe.ins, ScopedClock({None: tick_clock.global_clock}))
    si = probe.ins.sync_info
    waits = list(si.on_wait) if si is not None else []
    probe.ins.sync_info = mybir.SyncInfo(on_wait=waits[:1], on_update=[])
    name2sem = {s.name: s for s in self.sems.allocated().values()}
    for w in waits[1:]:
        nc.sync.nop(nofuse=True)._wait_ge(name2sem[w.ant_name], w.wait_value)
    nc.sync.drain()
    nc.all_engine_barrier()
    popped = nc._tile_sem_poison_stack.pop()
    assert popped is self._sem_poison
    nc.clear_and_free_semaphores(list(self.sems.allocated().values()))
    nc.all_engine_barrier()


TileContext._drain_and_barrier = _patched_drain_and_barrier

# ---------------------------------------------------------------------------

B, H, W, C = 8, 64, 64, 256
N = H * W
F32 = mybir.dt.float32
BF16 = mybir.dt.bfloat16
ALU = mybir.AluOpType
ACTF = mybir.ActivationFunctionType
BF = ml_dtypes.bfloat16


def _host_matrices():
    k = np.arange(64)
    j = np.arange(64)
    ang = np.pi * k[:, None] * (2 * j[None, :] + 1) / 128.0
    A = 2.0 * np.cos(ang)
    AT = A.T.astype(np.float32)                          # [h, k]
    u = np.where(k == 0, 1.0, 2.0)
    C1T = np.cos(ang)                                    # [k, i]
    S1T = np.sin(ang)
    C2T = u[:, None] * np.cos(ang) / 16384.0             # [l, j']
    S2T = u[:, None] * np.sin(ang) / 16384.0
    CS2 = np.concatenate([C2T, S2T], axis=1)             # [l, (cs j')]

    at2 = np.concatenate([AT, AT], axis=0).astype(BF)    # [128, 64]
    aw2 = np.zeros((128, 128), np.float32)
    aw2[:64, :64] = AT
    aw2[64:, 64:] = AT
    cs2c = np.zeros((128, 256), np.float32)
    cs2c[:64, :128] = CS2
    cs2c[64:, 128:] = CS2
    bc1 = np.zeros((128, 128), np.float32)
    bc1[:64, :64] = C1T
    bc1[64:, 64:] = C1T
    bs1 = np.zeros((128, 128), np.float32)
    bs1[:64, :64] = -S1T
    bs1[64:, 64:] = -S1T
    return (at2, aw2.astype(BF), cs2c.astype(BF), bc1.astype(BF), bs1.astype(BF))


_NC_CACHE = {}


def _build_nc(apply_gb):
    nc = bass.Bass(trn_type="TRN2")

    xr_d = nc.dram_tensor("xr", [128, 8192], BF16, kind="ExternalInput")
    at2_d = nc.dram_tensor("at2", [128, 64], BF16, kind="ExternalInput")
    aw2_d = nc.dram_tensor("aw2", [128, 128], BF16, kind="ExternalInput")
    cs2_d = nc.dram_tensor("cs2c", [128, 256], BF16, kind="ExternalInput")
    bc1_d = nc.dram_tensor("bc1", [128, 128], BF16, kind="ExternalInput")
    bs1_d = nc.dram_tensor("bs1", [128, 128], BF16, kind="ExternalInput")
    wfa_d = nc.dram_tensor("weffa", [128, 4096], BF16, kind="ExternalInput")
    wfb_d = nc.dram_tensor("weffb", [128, 4096], BF16, kind="ExternalInput")
    pja_d = nc.dram_tensor("pja", [128, 256], BF16, kind="ExternalInput")
    pjb_d = nc.dram_tensor("pjb", [128, 256], BF16, kind="ExternalInput")
    gb_d = nc.dram_tensor("gb", [2, 256], F32, kind="ExternalInput")
    y_d = nc.dram_tensor("y", [128, 8192], BF16, kind="ExternalOutput")

    with TileContext(nc) as tc:
        with (
            tc.tile_pool(name="sb", bufs=1) as sbp,
            tc.tile_pool(name="yb", bufs=2) as ybp,
            tc.tile_pool(name="dramp", bufs=1, space="DRAM") as dramp,
            tc.tile_pool(name="ps", bufs=8, space="PSUM") as ps,
            tc.tile_pool(name="small", bufs=8) as small,
        ):
            # ---- constants ----
            at2s = sbp.tile([128, 64], BF16, tag="at2")
            aw2s = sbp.tile([128, 128], BF16, tag="aw2")
            cs2s = sbp.tile([128, 256], BF16, tag="cs2")
            bc1s = sbp.tile([128, 128], BF16, tag="bc1")
            bs1s = sbp.tile([128, 128], BF16, tag="bs1")
            pja_s = sbp.tile([128, 256], BF16, tag="pja")
            pjb_s = sbp.tile([128, 256], BF16, tag="pjb")
            pjs = {0: pja_s, 1: pjb_s}
            nc.scalar.dma_start(out=at2s[:], in_=at2_d[:])
            nc.scalar.dma_start(out=aw2s[:], in_=aw2_d[:])
            nc.scalar.dma_start(out=cs2s[:], in_=cs2_d[:])
            nc.gpsimd.dma_start(out=bc1s[:], in_=bc1_d[:])
            nc.gpsimd.dma_start(out=bs1s[:], in_=bs1_d[:])
            nc.gpsimd.dma_start(out=pjs[0][:], in_=pja_d[:])
            nc.gpsimd.dma_start(out=pjs[1][:], in_=pjb_d[:])
            eps = sbp.tile([128, 1], F32, tag="eps")
            nc.vector.memset(eps[:], 1e-5)
            weff0 = sbp.tile([128, 4096], BF16, tag="wfa")
            weff1 = sbp.tile([128, 4096], BF16, tag="wfb")
            weff = {0: weff0, 1: weff1}
            if apply_gb:
                gt = sbp.tile([128, 256], F32, tag="gt")
                bt = sbp.tile([128, 256], F32, tag="bt")
                gb_ap = gb_d.ap()
                g_b = bass.AP(tensor=gb_ap.tensor, offset=0, ap=[[0, 128], [1, 256]])
                b_b = bass.AP(tensor=gb_ap.tensor, offset=256, ap=[[0, 128], [1, 256]])
                nc.sync.dma_start(out=gt[:], in_=g_b)
                nc.sync.dma_start(out=bt[:], in_=b_b)

            # ---- input load (4 chunks so F-h can start early) ----
            xq = sbp.tile([128, 8192], BF16, tag="xq")
            for q in range(4):
                qs = slice(q * 2048, (q + 1) * 2048)
                nc.sync.dma_start(out=xq[:, qs], in_=xr_d[:, qs])
            nc.scalar.dma_start(out=weff[0][:], in_=wfa_d[:])
            nc.scalar.dma_start(out=weff[1][:], in_=wfb_d[:])

            G0 = sbp.tile([128, 4096], BF16, tag="g0")
            G1 = sbp.tile([128, 4096], BF16, tag="g1")
            Z0 = sbp.tile([128, 4096], BF16, tag="z0")
            Z1 = sbp.tile([128, 4096], BF16, tag="z1")
            Vb0 = sbp.tile([128, 8192], BF16, tag="v0")
            Vb1 = sbp.tile([128, 8192], BF16, tag="v1")
            X20 = sbp.tile([128, 4096], BF16, tag="x20")
            X21 = sbp.tile([128, 4096], BF16, tag="x21")
            Xp0 = sbp.tile([128, 4096], BF16, tag="xp0")
            Xp1 = sbp.tile([128, 4096], BF16, tag="xp1")
            Dq0 = dramp.tile([128, 4096], BF16, tag="dq0")
            Dq1 = dramp.tile([128, 4096], BF16, tag="dq1")
            G = {0: G0, 1: G1}
            Z = {0: Z0, 1: Z1}
            Vb = {0: Vb0, 1: Vb1}
            X2 = {0: X20, 1: X21}
            Xp = {0: Xp0, 1: Xp1}
            Dq = {0: Dq0, 1: Dq1}

            # PSUM is only reachable from Vector (DVE) and Scalar (Act).
            cp_eng = [nc.scalar.copy, nc.vector.tensor_copy]

            def fh(g):
                """F-h group g: 8 c-pair chunks x both P, P-interleaved."""
                pt0 = ps.tile([128, 512], F32, tag="ps")
                pt1 = ps.tile([128, 512], F32, tag="ps")
                pts = {0: pt0, 1: pt1}
                for s in range(8):
                    jj = 8 * g + s
                    for P in range(2):
                        rows = slice(P * 64, (P + 1) * 64)
                        nc.tensor.matmul(
                            pts[P][:, s * 64:(s + 1) * 64],
                            xq[rows, jj * 128:(jj + 1) * 128],
                            at2s[rows, :], start=True, stop=True,
                        )
                gs = slice(g * 512, (g + 1) * 512)
                for P in range(2):
                    nc.scalar.copy(G[P][:, gs], pts[P][:])

            def fw(P, g):
                """F-w chunk g -> Z cols g*512.."""
                gs = slice(g * 512, (g + 1) * 512)
                pw = ps.tile([128, 512], F32, tag="ps")
                nc.tensor.matmul(pw[:], aw2s[:], G[P][:, gs], start=True, stop=True)
                nc.vector.tensor_mul(Z[P][:, gs], pw[:], weff[P][:, gs])

            def il(P, g):
                """I-l m = 4g..4g+3 -> Vb cols."""
                for mh in range(2):
                    pv = ps.tile([128, 512], F32, tag="ps")
                    for s in range(2):
                        m = 4 * g + 2 * mh + s
                        nc.tensor.matmul(
                            pv[:, s * 256:(s + 1) * 256],
                            Z[P][:, m * 128:(m + 1) * 128],
                            cs2s[:], start=True, stop=True,
                        )
                    m0 = 4 * g + 2 * mh
                    eng = nc.vector.tensor_copy if (4 * g + 2 * P + mh) % 3 == 0 else nc.scalar.copy
                    eng(Vb[P][:, m0 * 256:(m0 + 2) * 256], pv[:])

            def ik(P, r):
                """I-k round r: m in [4r, 4r+4) -> X2 cols r*512.."""
                vv = Vb[P][:].rearrange("p (m c2 cs j) -> p m c2 cs j",
                                        m=32, c2=2, cs=2, j=64)
                pk = ps.tile([128, 512], F32, tag="ps")
                nc.tensor.matmul(pk[:], bc1s[:], vv[:, 4 * r:4 * r + 4, :, 0, :],
                                 start=True, stop=False)
                nc.tensor.matmul(pk[:], bs1s[:], vv[:, 4 * r:4 * r + 4, :, 1, :],
                                 start=False, stop=True)
                rs = slice(r * 512, (r + 1) * 512)
                cp_eng[(r + P) % 2](X2[P][:, rs], pk[:])

            def pivot_store(P, half):
                hs = slice(half * 2048, (half + 1) * 2048)
                nc.sync.dma_start(out=Dq[P][:, hs], in_=X2[P][:, hs])

            def pivot_reload(P, mh):
                # Xp[cl=(mh, j2, m2, c2), (i, j')] <- Dq linear [(j2, i), (m, c2, j')]
                # half mh covers m = 16*mh + m2; lands on partition half mh*64.
                dq_ap = Dq[P][:]
                for j2 in range(2):
                    off = j2 * 64 * 4096 + mh * 16 * 128
                    in_ap = bass.AP(
                        tensor=dq_ap.tensor,
                        offset=dq_ap.offset + off,
                        ap=[[128, 16], [64, 2], [4096, 64], [1, 64]],
                    )
                    out_rows = slice(mh * 64 + j2 * 32, mh * 64 + j2 * 32 + 32)
                    eng = nc.sync if (j2 + P) % 2 == 0 else nc.scalar
                    eng.dma_start(out=Xp[P][out_rows, :], in_=in_ap)

            # ---- emit pipeline: phase-major so PE streams while drains lag.
            # I-k lags I-l by one group; pivot quarters issue as soon as their
            # two I-k rounds land, hiding the strided reload under compute. ----
            for g in range(8):
                fh(g)
            for g in range(8):
                for P in range(2):
                    fw(P, g)
            for g in range(8):
                for P in range(2):
                    il(P, g)
            for r in range(8):
                for P in range(2):
                    ik(P, r)
                if r == 3:
                    for P in range(2):
                        pivot_store(P, 0)
                    for P in range(2):
                        pivot_reload(P, 0)
            for P in range(2):
                pivot_store(P, 1)
            for P in range(2):
                pivot_reload(P, 1)

            # ---- proj + LN ----
            mvall = small.tile([128, 64], F32, tag="mvall")
            rstdall = small.tile([128, 32], F32, tag="rstdall")
            nmrall = small.tile([128, 32], F32, tag="nmrall")
            # proj: 4 accumulated K=64 matmuls per t-tile (P x mh). The mh=0
            # pair of the first PRE tiles is emitted right after I-k, so PE
            # fills the second pivot-half's reload window.
            ptys = {}
            ybufs = {}
            for t in range(32):
                if t % 2 == 0:
                    py2 = ps.tile([128, 512], F32, tag="ps")
                pysl = py2[:, (t % 2) * 256:(t % 2 + 1) * 256]
                nc.tensor.matmul(pysl, Xp[0][:, t * 128:(t + 1) * 128],
                                 pjs[0][:], start=True, stop=False)
                nc.tensor.matmul(pysl, Xp[1][:, t * 128:(t + 1) * 128],
                                 pjs[1][:], start=False, stop=True)
                stats = small.tile([128, 6], F32, tag="stats")
                nc.vector.bn_stats(out=stats[:], in_=pysl)
                nc.vector.bn_aggr(out=mvall[:, t * 2:(t + 1) * 2], in_=stats[:])
                ptys[t] = pysl
                if t % 8 == 7:
                    g0 = t - 7
                    gsl = slice(g0, g0 + 8)
                    mvv = mvall[:].rearrange("p (t x) -> p t x", x=2)
                    nc.scalar.activation(out=rstdall[:, gsl], in_=mvv[:, gsl, 1],
                                         func=ACTF.Sqrt, bias=eps[:], scale=1.0)
                    nc.vector.reciprocal(rstdall[:, gsl], rstdall[:, gsl])
                    nc.vector.tensor_tensor(out=nmrall[:, gsl], in0=mvv[:, gsl, 0],
                                            in1=rstdall[:, gsl], op=ALU.mult)
                    nc.vector.tensor_scalar_mul(nmrall[:, gsl], nmrall[:, gsl], -1.0)
                    yb = ybp.tile([128, 2048], BF16, tag="yb")
                    for t3 in range(g0, g0 + 8):
                        ysl = slice((t3 - g0) * 256, (t3 - g0 + 1) * 256)
                        if t3 % 4 == 3:
                            # DVE path: (py * rstd) + (-mu * rstd)
                            nc.vector.tensor_scalar(
                                out=yb[:, ysl], in0=ptys[t3],
                                scalar1=rstdall[:, t3:t3 + 1],
                                scalar2=nmrall[:, t3:t3 + 1],
                                op0=ALU.mult, op1=ALU.add)
                        else:
                            nc.scalar.activation(out=yb[:, ysl], in_=ptys[t3],
                                                 func=ACTF.Identity,
                                                 bias=nmrall[:, t3:t3 + 1],
                                                 scale=rstdall[:, t3:t3 + 1])
                        if apply_gb:
                            nc.vector.tensor_mul(yb[:, ysl], yb[:, ysl], gt[:])
                            nc.vector.tensor_add(yb[:, ysl], yb[:, ysl], bt[:])
                        del ptys[t3]
                    nc.gpsimd.dma_start(
                        out=y_d[:, g0 * 256:(g0 + 8) * 256], in_=yb[:])
                    ybufs[g0] = yb

    _split_multi_waits(nc)
    return nc


def _get_nc(apply_gb):
    key = bool(apply_gb)
    if key not in _NC_CACHE:
        _NC_CACHE[key] = _build_nc(key)
    return _NC_CACHE[key]


def _make_inputs(x, W_low, W_mid, W_high, proj_w, ln_g, ln_b):
    at2, aw2, cs2c, bc1, bs1 = _host_matrices()

    W_eff = W_high[0].copy()                       # [k, l, C]
    W_eff[:32, :32] += W_mid[0]
    W_eff[:16, :16] += W_low[0]
    weffs = []
    for P in range(2):
        wp = W_eff[:, :, P * 128:(P + 1) * 128].reshape(64, 64, 64, 2)
        weffs.append(np.ascontiguousarray(
            wp.transpose(3, 1, 2, 0).reshape(128, 4096).astype(BF)))

    # pjt_P[cl, d] = proj_w[d, ch],  cl = mh*64 + j2*32 + m2*2 + c2,
    # ch = P*128 + 4*(16*mh + m2) + 2*j2 + c2
    cl = np.arange(128)
    mh, j2, m2, c2 = cl // 64, (cl % 64) // 32, (cl % 32) // 2, cl % 2
    ch = 4 * (16 * mh + m2) + 2 * j2 + c2
    pjts = [np.ascontiguousarray(proj_w[:, P * 128 + ch].T.astype(BF))
            for P in range(2)]

    gb = np.stack([ln_g, ln_b]).astype(np.float32)
    consts = {"at2": at2, "aw2": aw2, "cs2c": cs2c, "bc1": bc1, "bs1": bs1,
              "weffa": weffs[0], "weffb": weffs[1],
              "pja": pjts[0], "pjb": pjts[1], "gb": gb}

    in_maps = []
    for b in range(B):
        mm = dict(consts)
        xr = x[b].reshape(64, 64, 2, 128).transpose(2, 0, 3, 1).reshape(128, 8192)
        mm["xr"] = np.ascontiguousarray(xr.astype(BF))
        in_maps.append(mm)
    return in_maps


def kernel(x, W_low, W_mid, W_high, proj_w, ln_g, ln_b):
    x = np.ascontiguousarray(np.asarray(x, dtype=np.float32))
    W_low = np.asarray(W_low, dtype=np.float32)
    W_mid = np.asarray(W_mid, dtype=np.float32)
    W_high = np.asarray(W_high, dtype=np.float32)
    proj_w = np.asarray(proj_w, dtype=np.float32)
    ln_g = np.asarray(ln_g, dtype=np.float32)
    ln_b = np.asarray(ln_b, dtype=np.float32)

    apply_gb = not (np.all(ln_g == 1.0) and np.all(ln_b == 0.0))
    in_maps = _make_inputs(x, W_low, W_mid, W_high, proj_w, ln_g, ln_b)
    nc = _get_nc(apply_gb)
    res = run_bass_kernel_spmd(nc, in_maps, core_ids=list(range(B)))

    out = np.empty((B, N, C), np.float32)
    for b in range(B):
        yc = np.asarray(res.results[b]["y"]).astype(np.float32)
        out[b] = yc.reshape(128, 32, 256).transpose(1, 0, 2).reshape(4096, 256)
    return out
